# revision 18
# baseline (speedup 1.0000x reference)
"""Trainium2 Bass kernel for nn_LiveNet (2-layer MLP: relu(x@W1+b1)@W2+b2).

Sharding: pure data-parallel over batch across 8 NeuronCores (no
collectives).  Each core computes y_i = relu(x_i @ W1 + b1) @ W2 + b2 for
its 512-row batch shard.

All matmul operands are bf16 (host-cast): same 1 cycle/row PE rate as
float32r but half the HBM traffic, so DMA stays well below the PE
roofline.  Accumulation is fp32 in PSUM.

GEMM1 (hT = relu(W1.T @ xT + b1)) is classical: per 128-row hid m-tile,
8 k-matmuls accumulate in PSUM and ScalarE applies bias+ReLU on evict to
a resident bf16 hT.  Warmup matmuls on a zeroed tile ramp the PE p-state
(full 2.4 GHz needs 3us of continuous execution) while startup DMAs land.

GEMM2 (y = h @ W2 + b2) uses one level of Strassen-Winograd: 7 products
of half-size blocks instead of 8, cutting PE time by 1/8.  The W2-side
block combinations (T1..T4) are precomputed on host and streamed; the
h-side combinations (S1..S4) run on the otherwise-idle VectorE as hT
tiles appear; the C-side recombination runs on VectorE against PSUM
directly (only M1/M5 are evicted, being multiply-used), with y writeback
staggered product-by-product.  b2 is injected via rank-1 PE updates into
M1 and M5 (every C block inherits exactly M1 + [C12/C22: M5's delta]).
y is written bf16 and upcast on host.
"""

import os
import sys

import numpy as np

for _p in ("/opt/trn_rl_repo", "/root/.axon_site/_ro/trn_rl_repo"):
    if os.path.isdir(_p) and _p not in sys.path:
        sys.path.append(_p)

import ml_dtypes

import concourse.bacc as bacc
import concourse.bass as bass
import concourse.tile as tile
from concourse import mybir
from concourse.bass_utils import run_bass_kernel_spmd

N_CORES = 8
B, N_IN, N_HID, N_OUT = 4096, 1024, 4096, 1024
BSH = B // N_CORES          # 512 batch rows per core
P = 128                     # SBUF partitions
KT1 = N_IN // P             # 8  k-tiles in GEMM1
MT1 = N_HID // P            # 32 m-tiles (hid) in GEMM1
KS = MT1 // 2               # 16 k-tiles per Strassen half (hid 2048)
NCH = 512                   # moving free dim per matmul / N_OUT half
HB = 256                    # batch half per Strassen block

F32 = mybir.dt.float32
BF16 = mybir.dt.bfloat16
RELU = mybir.ActivationFunctionType.Relu
COPY = mybir.ActivationFunctionType.Copy
BF = ml_dtypes.bfloat16

WARMUP = int(os.environ.get("K_WARMUP", "30"))   # 128-row p-state ramp matmuls

# Strassen product order: M1, M6, M5, M7, M2, M3, M4 (multiply-used products
# first so later products can be consumed straight out of PSUM).
PROD = ["M1", "M6", "M5", "M7", "M2", "M3", "M4"]
NP_ = len(PROD)


def build_nc(reps=1):
    nc = bacc.Bacc("TRN2", target_bir_lowering=False, debug=False,
                   num_devices=N_CORES)

    # xtr[p, k*512+j] = x[batch j, k*128+p]: per-partition contiguous.
    xtr = nc.declare_dram_parameter("xtr", [P, KT1 * BSH], BF16, isOutput=False)
    w1r = nc.declare_dram_parameter("w1r", [MT1, P, N_IN], BF16, isOutput=False)
    # W2-side Strassen operands, one [2048, 512] matrix per product.
    w2s = nc.declare_dram_parameter("w2s", [NP_, KS, P, NCH], BF16,
                                    isOutput=False)
    b1t = nc.declare_dram_parameter("b1t", [P, MT1], F32, isOutput=False)
    # b2v = [b2[:512], b2[512:]-b2[:512]] for the M1/M5 rank-1 bias updates.
    b2v = nc.declare_dram_parameter("b2v", [1, N_OUT], BF16, isOutput=False)
    y = nc.declare_dram_parameter("y", [BSH, N_OUT], BF16, isOutput=True)

    with tile.TileContext(nc) as tc:
        with (
            tc.tile_pool(name="const", bufs=1) as const,
            tc.tile_pool(name="xt", bufs=1) as xt_pool,
            tc.tile_pool(name="ht", bufs=1) as ht_pool,
            tc.tile_pool(name="w1", bufs=6) as w1_pool,
            tc.tile_pool(name="w2", bufs=40) as w2_pool,
            tc.tile_pool(name="scmb", bufs=1) as s_pool,
            tc.tile_pool(name="uacc", bufs=1) as u_pool,
            tc.tile_pool(name="yout", bufs=4) as y_pool,
            tc.tile_pool(name="ps", bufs=8, space=bass.MemorySpace.PSUM) as ps_pool,
        ):
            # --- PE warmup: ramp the p-state while startup DMAs land ---
            wz = const.tile([P, P], BF16)
            nc.vector.memset(wz[:], 0.0)
            wps = ps_pool.tile([P, NCH], F32, tag="ps", name="wps")
            for i in range(WARMUP):
                nc.tensor.matmul(wps[:, 0:P], wz[:], wz[:],
                                 start=(i == 0), stop=(i == WARMUP - 1))

            # --- startup DMAs, critical-path first.  xt k-tiles stream on
            # SP while the first W1 m-tiles issue in parallel from ACT (its
            # first evict comes much later). ---
            xt_sb = xt_pool.tile([P, KT1, BSH], BF16, tag="xt", name="xt_sb")
            w1_head = []
            for m in range(4):
                w1_sb = w1_pool.tile([P, N_IN], BF16, tag="w1", name="w1_sb")
                if m == 0:
                    # halves: the first matmul only waits on k-tiles 0..3
                    nc.scalar.dma_start(out=w1_sb[:, 0:N_IN // 2],
                                        in_=w1r[m, :, 0:N_IN // 2])
                    nc.scalar.dma_start(out=w1_sb[:, N_IN // 2:N_IN],
                                        in_=w1r[m, :, N_IN // 2:N_IN])
                else:
                    nc.scalar.dma_start(out=w1_sb[:], in_=w1r[m])
                w1_head.append(w1_sb)
            # xt k0 alone first so the opening matmul's moving operand lands
            # as early as possible, then the rest in k-pair chunks
            nc.sync.dma_start(out=xt_sb[:, 0, :], in_=xtr[:, 0:BSH])
            nc.sync.dma_start(out=xt_sb[:, 1, :], in_=xtr[:, BSH:2 * BSH])
            for q in range(3):
                k0 = 2 + 2 * q
                nc.sync.dma_start(
                    out=xt_sb[:, k0:k0 + 2, :],
                    in_=xtr[:, k0 * BSH:(k0 + 2) * BSH])

            b1_sb = const.tile([P, MT1], F32)
            nc.gpsimd.dma_start(out=b1_sb[:], in_=b1t[:])
            b2v_sb = const.tile([1, N_OUT], BF16)
            nc.gpsimd.dma_start(out=b2v_sb[:], in_=b2v[:])
            ones_sb = const.tile([1, P], BF16)
            nc.vector.memset(ones_sb[:], 1.0)
            # Prime ACT with the bias-load DMA wait so later evict
            # instructions (which already wait on the PE sem) don't exceed
            # the per-instruction sync-wait budget in walrus codegen.
            prime1 = const.tile([P, 1], F32)
            nc.scalar.activation(prime1[:], b1_sb[:, 0:1], COPY)
            prime2 = const.tile([1, 1], BF16)
            nc.vector.tensor_copy(prime2[:], b2v_sb[:, 0:1])

            for rep in range(reps):
                # hT resident: [128, 32, 512] bf16; hT[p, m, j] =
                # h[batch j, hid m*128+p].
                ht_sb = ht_pool.tile([P, MT1, BSH], BF16, tag="ht",
                                     name="ht_sb")
                # h-side Strassen combos: S1=A21+A22, S2=S1-A11,
                # S3=A11-A21, S4=A12-S2; each [128, 16, 256] bf16.
                s1 = s_pool.tile([P, KS, HB], BF16, tag="s1", name="s1")
                s2 = s_pool.tile([P, KS, HB], BF16, tag="s2", name="s2")
                s3 = s_pool.tile([P, KS, HB], BF16, tag="s3", name="s3")
                s4 = s_pool.tile([P, KS, HB], BF16, tag="s4", name="s4")

                # W2-side product operand stream (pool-throttled JIT).
                w2_tiles = []
                w2_iss = 0

                def issue_w2(cnt):
                    nonlocal w2_iss
                    for _ in range(cnt):
                        if w2_iss >= NP_ * KS:
                            return
                        i, kk = w2_iss // KS, w2_iss % KS
                        t = w2_pool.tile([P, NCH], BF16, tag="w2",
                                         name="w2t")
                        nc.sync.dma_start(out=t[:], in_=w2s[i, kk])
                        w2_tiles.append(t)
                        w2_iss += 1

                # ---- GEMM1: hT = relu(W1.T-tiled @ xT + b1) ----
                for m in range(MT1):
                    if m < 4:
                        w1_sb = w1_head[m]
                    else:
                        w1_sb = w1_pool.tile([P, N_IN], BF16, tag="w1",
                                             name="w1_sb")
                        nc.sync.dma_start(out=w1_sb[:], in_=w1r[m])
                    if rep == 0 and m >= 1:
                        issue_w2(2)
                    ps = ps_pool.tile([P, BSH], F32, tag="ps", name="ps")
                    for k in range(KT1):
                        nc.tensor.matmul(
                            ps[:],
                            w1_sb[:, k * P:(k + 1) * P],
                            xt_sb[:, k, :],
                            start=(k == 0),
                            stop=(k == KT1 - 1),
                        )
                    nc.scalar.activation(
                        ht_sb[:, m, :], ps[:], RELU, bias=b1_sb[:, m:m + 1],
                    )
                    if m >= KS:
                        kk = m - KS
                        a11 = ht_sb[:, kk, 0:HB]
                        a21 = ht_sb[:, kk, HB:BSH]
                        a12 = ht_sb[:, m, 0:HB]
                        a22 = ht_sb[:, m, HB:BSH]
                        nc.vector.tensor_add(s1[:, kk, :], a21, a22)
                        nc.vector.tensor_sub(s2[:, kk, :], s1[:, kk, :], a11)
                        nc.vector.tensor_sub(s4[:, kk, :], a12, s2[:, kk, :])
                        nc.vector.tensor_sub(s3[:, kk, :], a11, a21)
                issue_w2(NP_ * KS - w2_iss)

                # ---- GEMM2: one level of Strassen-Winograd ----
                # A-side operand for product name, k-tile kk, batch ptile p.
                def a_op(name, kk, p):
                    j0 = p * P
                    if name == "M1":          # A11
                        return ht_sb[:, kk, j0:j0 + P]
                    if name == "M2":          # A12
                        return ht_sb[:, KS + kk, j0:j0 + P]
                    if name == "M4":          # A22
                        return ht_sb[:, KS + kk, HB + j0:HB + j0 + P]
                    s = {"M6": s2, "M5": s1, "M7": s3, "M3": s4}[name]
                    return s[:, kk, j0:j0 + P]

                m1_sb = u_pool.tile([P, 2, NCH], F32, tag="m1", name="m1_sb")
                m5_sb = u_pool.tile([P, 2, NCH], F32, tag="m5", name="m5_sb")
                u2_sb = u_pool.tile([P, 2, NCH], F32, tag="u2", name="u2_sb")
                u3_sb = u_pool.tile([P, 2, NCH], F32, tag="u3", name="u3_sb")
                u4_sb = u_pool.tile([P, 2, NCH], F32, tag="u4", name="u4_sb")

                mprev = {}

                def emit_y(tile_src0, tile_src1, sub, rows0, col0, p,
                           via_sp=False):
                    """y[rows0+p*128 .., col0:col0+512] = src0 +/- src1."""
                    y_sb = y_pool.tile([P, NCH], BF16, tag="y", name="y_sb")
                    if sub:
                        nc.vector.tensor_sub(y_sb[:], tile_src0, tile_src1)
                    else:
                        nc.vector.tensor_add(y_sb[:], tile_src0, tile_src1)
                    eng = nc.sync if via_sp else nc.scalar
                    r = rows0 + p * P
                    eng.dma_start(out=y[r:r + P, col0:col0 + NCH],
                                  in_=y_sb[:])

                for i, name in enumerate(PROD):
                    pss = []
                    for p in range(2):
                        if i == NP_ - 1 and p == 1:
                            break  # handled below as two column halves
                        ps2 = ps_pool.tile([P, NCH], F32, tag="ps",
                                           name=f"ps_{name}_{p}")
                        biased = name in ("M1", "M5")
                        for kk in range(KS):
                            nc.tensor.matmul(
                                ps2[:],
                                a_op(name, kk, p),
                                w2_tiles[i * KS + kk][:],
                                start=(kk == 0),
                                stop=(kk == KS - 1) and not biased,
                            )
                        if biased:
                            off = 0 if name == "M1" else NCH
                            nc.tensor.matmul(
                                ps2[:], ones_sb[:], b2v_sb[:, off:off + NCH],
                                start=False, stop=True,
                            )
                        pss.append(ps2)

                    if name == "M1":
                        for p in range(2):
                            nc.scalar.activation(m1_sb[:, p, :], pss[p][:],
                                                 COPY)
                    elif name == "M5":
                        for p in range(2):
                            nc.scalar.activation(m5_sb[:, p, :], pss[p][:],
                                                 COPY)
                        for p in range(2):
                            nc.vector.tensor_add(u4_sb[:, p, :],
                                                 u2_sb[:, p, :],
                                                 m5_sb[:, p, :])
                    elif name == "M6":
                        for p in range(2):
                            nc.vector.tensor_add(u2_sb[:, p, :],
                                                 m1_sb[:, p, :], pss[p][:])
                    elif name == "M7":
                        for p in range(2):
                            nc.vector.tensor_add(u3_sb[:, p, :],
                                                 u2_sb[:, p, :], pss[p][:])
                        # C22 = U3 + M5 -> y[256:512, 512:1024]
                        for p in range(2):
                            emit_y(u3_sb[:, p, :], m5_sb[:, p, :], False,
                                   HB, NCH, p)
                    elif name == "M2":
                        # C11 = M1 + M2 -> y[0:256, 0:512]
                        for p in range(2):
                            emit_y(m1_sb[:, p, :], pss[p][:], False,
                                   0, 0, p)
                    elif name == "M3":
                        # C12 = U4 + M3 -> y[0:256, 512:1024]
                        for p in range(2):
                            emit_y(u4_sb[:, p, :], pss[p][:], False,
                                   0, NCH, p)
                    elif name == "M4":
                        # C21 = U3 - M4 -> y[256:512, 0:512].  ptile 0 is
                        # full-width; ptile 1 runs as two column halves in
                        # separate PSUM banks so the first half's evict+DMA
                        # hides under the second half's matmuls.
                        emit_y(u3_sb[:, 0, :], pss[0][:], True, HB, 0, 0,
                               via_sp=True)
                        HC = NCH // 2
                        for h in range(2):
                            c0 = h * HC
                            psh = ps_pool.tile([P, NCH], F32, tag="ps",
                                               name=f"ps_M4h{h}")
                            for kk in range(KS):
                                nc.tensor.matmul(
                                    psh[:, 0:HC],
                                    a_op(name, kk, 1),
                                    w2_tiles[i * KS + kk][:, c0:c0 + HC],
                                    start=(kk == 0),
                                    stop=(kk == KS - 1),
                                )
                            y_sb = y_pool.tile([P, HC], BF16, tag="yh",
                                               name="y_sbh")
                            nc.vector.tensor_sub(
                                y_sb[:], u3_sb[:, 1, c0:c0 + HC],
                                psh[:, 0:HC],
                            )
                            nc.sync.dma_start(
                                out=y[HB + P:BSH, c0:c0 + HC], in_=y_sb[:],
                            )
                    mprev[name] = pss
    nc.compile()
    return nc


def _prep_shared(W1, b1, W2, b2):
    W1 = np.ascontiguousarray(W1, dtype=np.float32)
    # w1r[m, p, k*128+c] = W1[k*128+p, m*128+c]
    w1r = np.ascontiguousarray(
        W1.reshape(KT1, P, MT1, P).transpose(2, 1, 0, 3)
    ).reshape(MT1, P, N_IN).astype(BF)
    b1t = np.ascontiguousarray(
        np.asarray(b1, dtype=np.float32).reshape(MT1, P).T
    )
    W2 = np.ascontiguousarray(W2, dtype=np.float32)
    B11, B12 = W2[:2048, :NCH], W2[:2048, NCH:]
    B21, B22 = W2[2048:, :NCH], W2[2048:, NCH:]
    T1 = B12 - B11
    T2 = B22 - T1
    T3 = B22 - B12
    T4 = T2 - B21
    bmats = {"M1": B11, "M6": T2, "M5": T1, "M7": T3, "M2": B21,
             "M3": B22, "M4": T4}
    w2s = np.stack([bmats[nm].reshape(KS, P, NCH) for nm in PROD]
                   ).astype(BF)
    b2 = np.asarray(b2, dtype=np.float32)
    b2v = np.concatenate([b2[:NCH], b2[NCH:] - b2[:NCH]])[None, :].astype(BF)
    return w1r, b1t, w2s, b2v


def kernel(x, W1, b1, W2, b2):
    x = np.ascontiguousarray(x, dtype=np.float32)
    w1r, b1t, w2s, b2v = _prep_shared(W1, b1, W2, b2)

    in_maps = []
    for i in range(N_CORES):
        xs = x[i * BSH:(i + 1) * BSH, :].T.astype(BF)          # [1024, 512]
        xtr_i = np.ascontiguousarray(
            xs.reshape(KT1, P, BSH).transpose(1, 0, 2)
        ).reshape(P, KT1 * BSH)
        in_maps.append(
            {"xtr": xtr_i, "w1r": w1r, "w2s": w2s, "b1t": b1t, "b2v": b2v}
        )

    nc = build_nc()
    res = run_bass_kernel_spmd(nc, in_maps, list(range(N_CORES)))
    y = np.concatenate(
        [np.asarray(res.results[i]["y"]) for i in range(N_CORES)], axis=0
    )
    return y.astype(np.float32)


if __name__ == "__main__":
    rng = np.random.default_rng(0)
    x = rng.standard_normal((B, N_IN), dtype=np.float32)
    W1 = rng.standard_normal((N_IN, N_HID), dtype=np.float32) / 32
    b1 = rng.standard_normal((N_HID,), dtype=np.float32) / 32
    W2 = rng.standard_normal((N_HID, N_OUT), dtype=np.float32) / 64
    b2 = rng.standard_normal((N_OUT,), dtype=np.float32) / 64
    y = kernel(x, W1, b1, W2, b2)
    h = np.maximum(x @ W1 + b1, 0)
    y_ref = h @ W2 + b2
    err = np.linalg.norm(y - y_ref) / np.linalg.norm(y_ref)
    print("rel_l2:", err)


# revision 19
# speedup vs baseline: 1.0019x; 1.0019x over previous
"""Trainium2 Bass kernel for nn_LiveNet (2-layer MLP: relu(x@W1+b1)@W2+b2).

Sharding: pure data-parallel over batch across 8 NeuronCores (no
collectives).  Each core computes y_i = relu(x_i @ W1 + b1) @ W2 + b2 for
its 512-row batch shard.

All matmul operands are bf16 (host-cast): same 1 cycle/row PE rate as
float32r but half the HBM traffic, so DMA stays well below the PE
roofline.  Accumulation is fp32 in PSUM.

GEMM1 (hT = relu(W1.T @ xT + b1)) is classical: per 128-row hid m-tile,
8 k-matmuls accumulate in PSUM and ScalarE applies bias+ReLU on evict to
a resident bf16 hT.  Warmup matmuls on a zeroed tile ramp the PE p-state
(full 2.4 GHz needs 3us of continuous execution) while startup DMAs land.

GEMM2 (y = h @ W2 + b2) uses one level of Strassen-Winograd: 7 products
of half-size blocks instead of 8, cutting PE time by 1/8.  The W2-side
block combinations (T1..T4) are precomputed on host and streamed; the
h-side combinations (S1..S4) run on the otherwise-idle VectorE as hT
tiles appear; the C-side recombination runs on VectorE against PSUM
directly (only M1/M5 are evicted, being multiply-used), with y writeback
staggered product-by-product.  b2 is injected via rank-1 PE updates into
M1 and M5 (every C block inherits exactly M1 + [C12/C22: M5's delta]).
y is written bf16 and upcast on host.
"""

import os
import sys

import numpy as np

for _p in ("/opt/trn_rl_repo", "/root/.axon_site/_ro/trn_rl_repo"):
    if os.path.isdir(_p) and _p not in sys.path:
        sys.path.append(_p)

import ml_dtypes

import concourse.bacc as bacc
import concourse.bass as bass
import concourse.tile as tile
from concourse import mybir
from concourse.bass_utils import run_bass_kernel_spmd

N_CORES = 8
B, N_IN, N_HID, N_OUT = 4096, 1024, 4096, 1024
BSH = B // N_CORES          # 512 batch rows per core
P = 128                     # SBUF partitions
KT1 = N_IN // P             # 8  k-tiles in GEMM1
MT1 = N_HID // P            # 32 m-tiles (hid) in GEMM1
KS = MT1 // 2               # 16 k-tiles per Strassen half (hid 2048)
NCH = 512                   # moving free dim per matmul / N_OUT half
HB = 256                    # batch half per Strassen block

F32 = mybir.dt.float32
BF16 = mybir.dt.bfloat16
RELU = mybir.ActivationFunctionType.Relu
COPY = mybir.ActivationFunctionType.Copy
BF = ml_dtypes.bfloat16

WARMUP = int(os.environ.get("K_WARMUP", "30"))   # 128-row p-state ramp matmuls

# Strassen product order: M1, M6, M5, M7, M2, M3, M4 (multiply-used products
# first so later products can be consumed straight out of PSUM).
PROD = ["M1", "M6", "M5", "M7", "M2", "M3", "M4"]
NP_ = len(PROD)


def build_nc(reps=1):
    nc = bacc.Bacc("TRN2", target_bir_lowering=False, debug=False,
                   num_devices=N_CORES)

    # xtr[p, k*512+j] = x[batch j, k*128+p]: per-partition contiguous.
    xtr = nc.declare_dram_parameter("xtr", [P, KT1 * BSH], BF16, isOutput=False)
    w1r = nc.declare_dram_parameter("w1r", [MT1, P, N_IN], BF16, isOutput=False)
    # W2-side Strassen operands, one [2048, 512] matrix per product.
    w2s = nc.declare_dram_parameter("w2s", [NP_, KS, P, NCH], BF16,
                                    isOutput=False)
    b1t = nc.declare_dram_parameter("b1t", [P, MT1], F32, isOutput=False)
    # b2v = [b2[:512], b2[512:]-b2[:512]] for the M1/M5 rank-1 bias updates.
    b2v = nc.declare_dram_parameter("b2v", [1, N_OUT], BF16, isOutput=False)
    y = nc.declare_dram_parameter("y", [BSH, N_OUT], BF16, isOutput=True)

    with tile.TileContext(nc) as tc:
        with (
            tc.tile_pool(name="const", bufs=1) as const,
            tc.tile_pool(name="xt", bufs=1) as xt_pool,
            tc.tile_pool(name="ht", bufs=1) as ht_pool,
            tc.tile_pool(name="w1", bufs=6) as w1_pool,
            tc.tile_pool(name="w2", bufs=40) as w2_pool,
            tc.tile_pool(name="scmb", bufs=1) as s_pool,
            tc.tile_pool(name="uacc", bufs=1) as u_pool,
            tc.tile_pool(name="yout", bufs=4) as y_pool,
            tc.tile_pool(name="ps", bufs=8, space=bass.MemorySpace.PSUM) as ps_pool,
        ):
            # --- PE warmup: ramp the p-state while startup DMAs land ---
            wz = const.tile([P, P], BF16)
            nc.vector.memset(wz[:], 0.0)
            wps = ps_pool.tile([P, NCH], F32, tag="ps", name="wps")
            for i in range(WARMUP):
                nc.tensor.matmul(wps[:, 0:P], wz[:], wz[:],
                                 start=(i == 0), stop=(i == WARMUP - 1))

            # --- startup DMAs, critical-path first.  xt k-tiles stream on
            # SP while the first W1 m-tiles issue in parallel from ACT (its
            # first evict comes much later). ---
            xt_sb = xt_pool.tile([P, KT1, BSH], BF16, tag="xt", name="xt_sb")
            w1_head = []
            for m in range(4):
                w1_sb = w1_pool.tile([P, N_IN], BF16, tag="w1", name="w1_sb")
                if m == 0:
                    # halves: the first matmul only waits on k-tiles 0..3
                    nc.scalar.dma_start(out=w1_sb[:, 0:N_IN // 2],
                                        in_=w1r[m, :, 0:N_IN // 2])
                    nc.scalar.dma_start(out=w1_sb[:, N_IN // 2:N_IN],
                                        in_=w1r[m, :, N_IN // 2:N_IN])
                else:
                    nc.scalar.dma_start(out=w1_sb[:], in_=w1r[m])
                w1_head.append(w1_sb)
            for q in range(4):
                nc.sync.dma_start(
                    out=xt_sb[:, 2 * q:2 * q + 2, :],
                    in_=xtr[:, 2 * q * BSH:(2 * q + 2) * BSH])

            b1_sb = const.tile([P, MT1], F32)
            nc.gpsimd.dma_start(out=b1_sb[:], in_=b1t[:])
            b2v_sb = const.tile([1, N_OUT], BF16)
            nc.gpsimd.dma_start(out=b2v_sb[:], in_=b2v[:])
            ones_sb = const.tile([1, P], BF16)
            nc.vector.memset(ones_sb[:], 1.0)
            # Prime ACT with the bias-load DMA wait so later evict
            # instructions (which already wait on the PE sem) don't exceed
            # the per-instruction sync-wait budget in walrus codegen.
            prime1 = const.tile([P, 1], F32)
            nc.scalar.activation(prime1[:], b1_sb[:, 0:1], COPY)
            prime2 = const.tile([1, 1], BF16)
            nc.vector.tensor_copy(prime2[:], b2v_sb[:, 0:1])

            for rep in range(reps):
                # hT resident: [128, 32, 512] bf16; hT[p, m, j] =
                # h[batch j, hid m*128+p].
                ht_sb = ht_pool.tile([P, MT1, BSH], BF16, tag="ht",
                                     name="ht_sb")
                # h-side Strassen combos: S1=A21+A22, S2=S1-A11,
                # S3=A11-A21, S4=A12-S2; each [128, 16, 256] bf16.
                s1 = s_pool.tile([P, KS, HB], BF16, tag="s1", name="s1")
                s2 = s_pool.tile([P, KS, HB], BF16, tag="s2", name="s2")
                s3 = s_pool.tile([P, KS, HB], BF16, tag="s3", name="s3")
                s4 = s_pool.tile([P, KS, HB], BF16, tag="s4", name="s4")

                # W2-side product operand stream (pool-throttled JIT).
                w2_tiles = []
                w2_iss = 0

                def issue_w2(cnt):
                    nonlocal w2_iss
                    for _ in range(cnt):
                        if w2_iss >= NP_ * KS:
                            return
                        i, kk = w2_iss // KS, w2_iss % KS
                        t = w2_pool.tile([P, NCH], BF16, tag="w2",
                                         name="w2t")
                        nc.sync.dma_start(out=t[:], in_=w2s[i, kk])
                        w2_tiles.append(t)
                        w2_iss += 1

                # ---- GEMM1: hT = relu(W1.T-tiled @ xT + b1) ----
                for m in range(MT1):
                    if m < 4:
                        w1_sb = w1_head[m]
                    else:
                        w1_sb = w1_pool.tile([P, N_IN], BF16, tag="w1",
                                             name="w1_sb")
                        nc.sync.dma_start(out=w1_sb[:], in_=w1r[m])
                    if rep == 0 and m >= 1:
                        issue_w2(2)
                    ps = ps_pool.tile([P, BSH], F32, tag="ps", name="ps")
                    for k in range(KT1):
                        nc.tensor.matmul(
                            ps[:],
                            w1_sb[:, k * P:(k + 1) * P],
                            xt_sb[:, k, :],
                            start=(k == 0),
                            stop=(k == KT1 - 1),
                        )
                    nc.scalar.activation(
                        ht_sb[:, m, :], ps[:], RELU, bias=b1_sb[:, m:m + 1],
                    )
                    if m >= KS:
                        kk = m - KS
                        a11 = ht_sb[:, kk, 0:HB]
                        a21 = ht_sb[:, kk, HB:BSH]
                        a12 = ht_sb[:, m, 0:HB]
                        a22 = ht_sb[:, m, HB:BSH]
                        nc.vector.tensor_add(s1[:, kk, :], a21, a22)
                        nc.vector.tensor_sub(s2[:, kk, :], s1[:, kk, :], a11)
                        nc.vector.tensor_sub(s4[:, kk, :], a12, s2[:, kk, :])
                        nc.vector.tensor_sub(s3[:, kk, :], a11, a21)
                issue_w2(NP_ * KS - w2_iss)

                # ---- GEMM2: one level of Strassen-Winograd ----
                # A-side operand for product name, k-tile kk, batch ptile p.
                def a_op(name, kk, p):
                    j0 = p * P
                    if name == "M1":          # A11
                        return ht_sb[:, kk, j0:j0 + P]
                    if name == "M2":          # A12
                        return ht_sb[:, KS + kk, j0:j0 + P]
                    if name == "M4":          # A22
                        return ht_sb[:, KS + kk, HB + j0:HB + j0 + P]
                    s = {"M6": s2, "M5": s1, "M7": s3, "M3": s4}[name]
                    return s[:, kk, j0:j0 + P]

                m1_sb = u_pool.tile([P, 2, NCH], F32, tag="m1", name="m1_sb")
                m5_sb = u_pool.tile([P, 2, NCH], F32, tag="m5", name="m5_sb")
                u2_sb = u_pool.tile([P, 2, NCH], F32, tag="u2", name="u2_sb")
                u3_sb = u_pool.tile([P, 2, NCH], F32, tag="u3", name="u3_sb")
                u4_sb = u_pool.tile([P, 2, NCH], F32, tag="u4", name="u4_sb")

                mprev = {}

                def emit_y(tile_src0, tile_src1, sub, rows0, col0, p,
                           via_sp=False):
                    """y[rows0+p*128 .., col0:col0+512] = src0 +/- src1."""
                    y_sb = y_pool.tile([P, NCH], BF16, tag="y", name="y_sb")
                    if sub:
                        nc.vector.tensor_sub(y_sb[:], tile_src0, tile_src1)
                    else:
                        nc.vector.tensor_add(y_sb[:], tile_src0, tile_src1)
                    eng = nc.sync if via_sp else nc.scalar
                    r = rows0 + p * P
                    eng.dma_start(out=y[r:r + P, col0:col0 + NCH],
                                  in_=y_sb[:])

                for i, name in enumerate(PROD):
                    pss = []
                    for p in range(2):
                        if i == NP_ - 1 and p == 1:
                            break  # handled below as two column halves
                        ps2 = ps_pool.tile([P, NCH], F32, tag="ps",
                                           name=f"ps_{name}_{p}")
                        biased = name in ("M1", "M5")
                        for kk in range(KS):
                            nc.tensor.matmul(
                                ps2[:],
                                a_op(name, kk, p),
                                w2_tiles[i * KS + kk][:],
                                start=(kk == 0),
                                stop=(kk == KS - 1) and not biased,
                            )
                        if biased:
                            off = 0 if name == "M1" else NCH
                            nc.tensor.matmul(
                                ps2[:], ones_sb[:], b2v_sb[:, off:off + NCH],
                                start=False, stop=True,
                            )
                        pss.append(ps2)

                    if name == "M1":
                        for p in range(2):
                            nc.scalar.activation(m1_sb[:, p, :], pss[p][:],
                                                 COPY)
                    elif name == "M5":
                        for p in range(2):
                            nc.scalar.activation(m5_sb[:, p, :], pss[p][:],
                                                 COPY)
                        for p in range(2):
                            nc.vector.tensor_add(u4_sb[:, p, :],
                                                 u2_sb[:, p, :],
                                                 m5_sb[:, p, :])
                    elif name == "M6":
                        for p in range(2):
                            nc.vector.tensor_add(u2_sb[:, p, :],
                                                 m1_sb[:, p, :], pss[p][:])
                    elif name == "M7":
                        for p in range(2):
                            nc.vector.tensor_add(u3_sb[:, p, :],
                                                 u2_sb[:, p, :], pss[p][:])
                        # C22 = U3 + M5 -> y[256:512, 512:1024]
                        for p in range(2):
                            emit_y(u3_sb[:, p, :], m5_sb[:, p, :], False,
                                   HB, NCH, p)
                    elif name == "M2":
                        # C11 = M1 + M2 -> y[0:256, 0:512]
                        for p in range(2):
                            emit_y(m1_sb[:, p, :], pss[p][:], False,
                                   0, 0, p)
                    elif name == "M3":
                        # C12 = U4 + M3 -> y[0:256, 512:1024]
                        for p in range(2):
                            emit_y(u4_sb[:, p, :], pss[p][:], False,
                                   0, NCH, p)
                    elif name == "M4":
                        # C21 = U3 - M4 -> y[256:512, 0:512].  ptile 0 is
                        # full-width; ptile 1 runs as two column halves in
                        # separate PSUM banks so the first half's evict+DMA
                        # hides under the second half's matmuls.
                        emit_y(u3_sb[:, 0, :], pss[0][:], True, HB, 0, 0,
                               via_sp=True)
                        HC = NCH // 2
                        for h in range(2):
                            c0 = h * HC
                            psh = ps_pool.tile([P, NCH], F32, tag="ps",
                                               name=f"ps_M4h{h}")
                            for kk in range(KS):
                                nc.tensor.matmul(
                                    psh[:, 0:HC],
                                    a_op(name, kk, 1),
                                    w2_tiles[i * KS + kk][:, c0:c0 + HC],
                                    start=(kk == 0),
                                    stop=(kk == KS - 1),
                                )
                            y_sb = y_pool.tile([P, HC], BF16, tag="yh",
                                               name="y_sbh")
                            nc.vector.tensor_sub(
                                y_sb[:], u3_sb[:, 1, c0:c0 + HC],
                                psh[:, 0:HC],
                            )
                            nc.sync.dma_start(
                                out=y[HB + P:BSH, c0:c0 + HC], in_=y_sb[:],
                            )
                    mprev[name] = pss
    nc.compile()
    return nc


def _prep_shared(W1, b1, W2, b2):
    W1 = np.ascontiguousarray(W1, dtype=np.float32)
    # w1r[m, p, k*128+c] = W1[k*128+p, m*128+c]
    w1r = np.ascontiguousarray(
        W1.reshape(KT1, P, MT1, P).transpose(2, 1, 0, 3)
    ).reshape(MT1, P, N_IN).astype(BF)
    b1t = np.ascontiguousarray(
        np.asarray(b1, dtype=np.float32).reshape(MT1, P).T
    )
    W2 = np.ascontiguousarray(W2, dtype=np.float32)
    B11, B12 = W2[:2048, :NCH], W2[:2048, NCH:]
    B21, B22 = W2[2048:, :NCH], W2[2048:, NCH:]
    T1 = B12 - B11
    T2 = B22 - T1
    T3 = B22 - B12
    T4 = T2 - B21
    bmats = {"M1": B11, "M6": T2, "M5": T1, "M7": T3, "M2": B21,
             "M3": B22, "M4": T4}
    w2s = np.stack([bmats[nm].reshape(KS, P, NCH) for nm in PROD]
                   ).astype(BF)
    b2 = np.asarray(b2, dtype=np.float32)
    b2v = np.concatenate([b2[:NCH], b2[NCH:] - b2[:NCH]])[None, :].astype(BF)
    return w1r, b1t, w2s, b2v


def kernel(x, W1, b1, W2, b2):
    x = np.ascontiguousarray(x, dtype=np.float32)
    w1r, b1t, w2s, b2v = _prep_shared(W1, b1, W2, b2)

    in_maps = []
    for i in range(N_CORES):
        xs = x[i * BSH:(i + 1) * BSH, :].T.astype(BF)          # [1024, 512]
        xtr_i = np.ascontiguousarray(
            xs.reshape(KT1, P, BSH).transpose(1, 0, 2)
        ).reshape(P, KT1 * BSH)
        in_maps.append(
            {"xtr": xtr_i, "w1r": w1r, "w2s": w2s, "b1t": b1t, "b2v": b2v}
        )

    nc = build_nc()
    res = run_bass_kernel_spmd(nc, in_maps, list(range(N_CORES)))
    y = np.concatenate(
        [np.asarray(res.results[i]["y"]) for i in range(N_CORES)], axis=0
    )
    return y.astype(np.float32)


if __name__ == "__main__":
    rng = np.random.default_rng(0)
    x = rng.standard_normal((B, N_IN), dtype=np.float32)
    W1 = rng.standard_normal((N_IN, N_HID), dtype=np.float32) / 32
    b1 = rng.standard_normal((N_HID,), dtype=np.float32) / 32
    W2 = rng.standard_normal((N_HID, N_OUT), dtype=np.float32) / 64
    b2 = rng.standard_normal((N_OUT,), dtype=np.float32) / 64
    y = kernel(x, W1, b1, W2, b2)
    h = np.maximum(x @ W1 + b1, 0)
    y_ref = h @ W2 + b2
    err = np.linalg.norm(y - y_ref) / np.linalg.norm(y_ref)
    print("rel_l2:", err)


# revision 20
# speedup vs baseline: 1.0033x; 1.0014x over previous
"""Trainium2 Bass kernel for nn_LiveNet (2-layer MLP: relu(x@W1+b1)@W2+b2).

Sharding: pure data-parallel over batch across 8 NeuronCores (no
collectives).  Each core computes y_i = relu(x_i @ W1 + b1) @ W2 + b2 for
its 512-row batch shard.

All matmul operands are bf16 (host-cast): same 1 cycle/row PE rate as
float32r but half the HBM traffic, so DMA stays well below the PE
roofline.  Accumulation is fp32 in PSUM.

GEMM1 (hT = relu(W1.T @ xT + b1)) is classical: per 128-row hid m-tile,
8 k-matmuls accumulate in PSUM and ScalarE applies bias+ReLU on evict to
a resident bf16 hT.  Warmup matmuls on a zeroed tile ramp the PE p-state
(full 2.4 GHz needs 3us of continuous execution) while startup DMAs land.

GEMM2 (y = h @ W2 + b2) uses one level of Strassen-Winograd: 7 products
of half-size blocks instead of 8, cutting PE time by 1/8.  The W2-side
block combinations (T1..T4) are precomputed on host and streamed; the
h-side combinations (S1..S4) run on the otherwise-idle VectorE as hT
tiles appear; the C-side recombination runs on VectorE against PSUM
directly (only M1/M5 are evicted, being multiply-used), with y writeback
staggered product-by-product.  b2 is injected via rank-1 PE updates into
M1 and M5 (every C block inherits exactly M1 + [C12/C22: M5's delta]).
y is written bf16 and upcast on host.
"""

import os
import sys

import numpy as np

for _p in ("/opt/trn_rl_repo", "/root/.axon_site/_ro/trn_rl_repo"):
    if os.path.isdir(_p) and _p not in sys.path:
        sys.path.append(_p)

import ml_dtypes

import concourse.bacc as bacc
import concourse.bass as bass
import concourse.tile as tile
from concourse import mybir
from concourse.bass_utils import run_bass_kernel_spmd

N_CORES = 8
B, N_IN, N_HID, N_OUT = 4096, 1024, 4096, 1024
BSH = B // N_CORES          # 512 batch rows per core
P = 128                     # SBUF partitions
KT1 = N_IN // P             # 8  k-tiles in GEMM1
MT1 = N_HID // P            # 32 m-tiles (hid) in GEMM1
KS = MT1 // 2               # 16 k-tiles per Strassen half (hid 2048)
NCH = 512                   # moving free dim per matmul / N_OUT half
HB = 256                    # batch half per Strassen block

F32 = mybir.dt.float32
BF16 = mybir.dt.bfloat16
RELU = mybir.ActivationFunctionType.Relu
COPY = mybir.ActivationFunctionType.Copy
BF = ml_dtypes.bfloat16

WARMUP = int(os.environ.get("K_WARMUP", "30"))   # 128-row p-state ramp matmuls

# Strassen product order: M1, M6, M5, M7, M2, M3, M4 (multiply-used products
# first so later products can be consumed straight out of PSUM).
PROD = ["M1", "M6", "M5", "M7", "M2", "M3", "M4"]
NP_ = len(PROD)


def build_nc(reps=1):
    nc = bacc.Bacc("TRN2", target_bir_lowering=False, debug=False,
                   num_devices=N_CORES)

    # xtr[p, k*512+j] = x[batch j, k*128+p]: per-partition contiguous.
    xtr = nc.declare_dram_parameter("xtr", [P, KT1 * BSH], BF16, isOutput=False)
    w1r = nc.declare_dram_parameter("w1r", [MT1, P, N_IN], BF16, isOutput=False)
    # W2-side Strassen operands, one [2048, 512] matrix per product.
    w2s = nc.declare_dram_parameter("w2s", [NP_, KS, P, NCH], BF16,
                                    isOutput=False)
    b1t = nc.declare_dram_parameter("b1t", [P, MT1], F32, isOutput=False)
    # b2v = [b2[:512], b2[512:]-b2[:512]] for the M1/M5 rank-1 bias updates.
    b2v = nc.declare_dram_parameter("b2v", [1, N_OUT], BF16, isOutput=False)
    y = nc.declare_dram_parameter("y", [BSH, N_OUT], BF16, isOutput=True)

    with tile.TileContext(nc) as tc:
        with (
            tc.tile_pool(name="const", bufs=1) as const,
            tc.tile_pool(name="xt", bufs=1) as xt_pool,
            tc.tile_pool(name="ht", bufs=1) as ht_pool,
            tc.tile_pool(name="w1", bufs=6) as w1_pool,
            tc.tile_pool(name="w2", bufs=40) as w2_pool,
            tc.tile_pool(name="scmb", bufs=1) as s_pool,
            tc.tile_pool(name="uacc", bufs=1) as u_pool,
            tc.tile_pool(name="yout", bufs=4) as y_pool,
            tc.tile_pool(name="ps", bufs=8, space=bass.MemorySpace.PSUM) as ps_pool,
        ):
            # --- PE warmup: ramp the p-state while startup DMAs land ---
            wz = const.tile([P, P], BF16)
            nc.vector.memset(wz[:], 0.0)
            wps = ps_pool.tile([P, NCH], F32, tag="ps", name="wps")
            for i in range(WARMUP):
                nc.tensor.matmul(wps[:, 0:P], wz[:], wz[:],
                                 start=(i == 0), stop=(i == WARMUP - 1))

            # --- startup DMAs, critical-path first.  xt k-tiles stream on
            # SP while the first W1 m-tiles issue in parallel from ACT (its
            # first evict comes much later). ---
            xt_sb = xt_pool.tile([P, KT1, BSH], BF16, tag="xt", name="xt_sb")
            w1_head = []
            for m in range(4):
                w1_sb = w1_pool.tile([P, N_IN], BF16, tag="w1", name="w1_sb")
                if m == 0:
                    # halves: the first matmul only waits on k-tiles 0..3
                    nc.scalar.dma_start(out=w1_sb[:, 0:N_IN // 2],
                                        in_=w1r[m, :, 0:N_IN // 2])
                    nc.scalar.dma_start(out=w1_sb[:, N_IN // 2:N_IN],
                                        in_=w1r[m, :, N_IN // 2:N_IN])
                else:
                    nc.scalar.dma_start(out=w1_sb[:], in_=w1r[m])
                w1_head.append(w1_sb)
            for q in range(4):
                nc.sync.dma_start(
                    out=xt_sb[:, 2 * q:2 * q + 2, :],
                    in_=xtr[:, 2 * q * BSH:(2 * q + 2) * BSH])

            b1_sb = const.tile([P, MT1], F32)
            nc.gpsimd.dma_start(out=b1_sb[:], in_=b1t[:])
            b2v_sb = const.tile([1, N_OUT], BF16)
            nc.gpsimd.dma_start(out=b2v_sb[:], in_=b2v[:])
            ones_sb = const.tile([1, P], BF16)
            nc.vector.memset(ones_sb[:], 1.0)
            # Prime ACT with the bias-load DMA wait so later evict
            # instructions (which already wait on the PE sem) don't exceed
            # the per-instruction sync-wait budget in walrus codegen.
            prime1 = const.tile([P, 1], F32)
            nc.scalar.activation(prime1[:], b1_sb[:, 0:1], COPY)
            prime2 = const.tile([1, 1], BF16)
            nc.vector.tensor_copy(prime2[:], b2v_sb[:, 0:1])

            for rep in range(reps):
                # hT resident: [128, 32, 512] bf16; hT[p, m, j] =
                # h[batch j, hid m*128+p].
                ht_sb = ht_pool.tile([P, MT1, BSH], BF16, tag="ht",
                                     name="ht_sb")
                # h-side Strassen combos: S1=A21+A22, S2=S1-A11,
                # S3=A11-A21, S4=A12-S2; each [128, 16, 256] bf16.
                s1 = s_pool.tile([P, KS, HB], BF16, tag="s1", name="s1")
                s2 = s_pool.tile([P, KS, HB], BF16, tag="s2", name="s2")
                s3 = s_pool.tile([P, KS, HB], BF16, tag="s3", name="s3")
                s4 = s_pool.tile([P, KS, HB], BF16, tag="s4", name="s4")

                # W2-side product operand stream (pool-throttled JIT).
                w2_tiles = []
                w2_iss = 0

                def issue_w2(cnt):
                    nonlocal w2_iss
                    for _ in range(cnt):
                        if w2_iss >= NP_ * KS:
                            return
                        i, kk = w2_iss // KS, w2_iss % KS
                        t = w2_pool.tile([P, NCH], BF16, tag="w2",
                                         name="w2t")
                        nc.sync.dma_start(out=t[:], in_=w2s[i, kk])
                        w2_tiles.append(t)
                        w2_iss += 1

                # ---- GEMM1: hT = relu(W1.T-tiled @ xT + b1) ----
                for m in range(MT1):
                    if m < 4:
                        w1_sb = w1_head[m]
                    else:
                        w1_sb = w1_pool.tile([P, N_IN], BF16, tag="w1",
                                             name="w1_sb")
                        nc.sync.dma_start(out=w1_sb[:], in_=w1r[m])
                    if rep == 0 and m >= 1:
                        issue_w2(2)
                    ps = ps_pool.tile([P, BSH], F32, tag="ps", name="ps")
                    for k in range(KT1):
                        nc.tensor.matmul(
                            ps[:],
                            w1_sb[:, k * P:(k + 1) * P],
                            xt_sb[:, k, :],
                            start=(k == 0),
                            stop=(k == KT1 - 1),
                        )
                    nc.scalar.activation(
                        ht_sb[:, m, :], ps[:], RELU, bias=b1_sb[:, m:m + 1],
                    )
                    if m >= KS:
                        kk = m - KS
                        a11 = ht_sb[:, kk, 0:HB]
                        a21 = ht_sb[:, kk, HB:BSH]
                        a12 = ht_sb[:, m, 0:HB]
                        a22 = ht_sb[:, m, HB:BSH]
                        nc.vector.tensor_add(s1[:, kk, :], a21, a22)
                        nc.vector.tensor_sub(s2[:, kk, :], s1[:, kk, :], a11)
                        nc.vector.tensor_sub(s4[:, kk, :], a12, s2[:, kk, :])
                        nc.vector.tensor_sub(s3[:, kk, :], a11, a21)
                issue_w2(NP_ * KS - w2_iss)

                # ---- GEMM2: one level of Strassen-Winograd ----
                # A-side operand for product name, k-tile kk, batch ptile p.
                def a_op(name, kk, p):
                    j0 = p * P
                    if name == "M1":          # A11
                        return ht_sb[:, kk, j0:j0 + P]
                    if name == "M2":          # A12
                        return ht_sb[:, KS + kk, j0:j0 + P]
                    if name == "M4":          # A22
                        return ht_sb[:, KS + kk, HB + j0:HB + j0 + P]
                    s = {"M6": s2, "M5": s1, "M7": s3, "M3": s4}[name]
                    return s[:, kk, j0:j0 + P]

                m1_sb = u_pool.tile([P, 2, NCH], F32, tag="m1", name="m1_sb")
                m5_sb = u_pool.tile([P, 2, NCH], F32, tag="m5", name="m5_sb")
                u2_sb = u_pool.tile([P, 2, NCH], F32, tag="u2", name="u2_sb")
                u3_sb = u_pool.tile([P, 2, NCH], F32, tag="u3", name="u3_sb")
                u4_sb = u_pool.tile([P, 2, NCH], F32, tag="u4", name="u4_sb")

                mprev = {}

                def emit_y(tile_src0, tile_src1, sub, rows0, col0, p,
                           via_sp=False):
                    """y[rows0+p*128 .., col0:col0+512] = src0 +/- src1."""
                    y_sb = y_pool.tile([P, NCH], BF16, tag="y", name="y_sb")
                    if sub:
                        nc.vector.tensor_sub(y_sb[:], tile_src0, tile_src1)
                    else:
                        nc.vector.tensor_add(y_sb[:], tile_src0, tile_src1)
                    eng = nc.sync if via_sp else nc.scalar
                    r = rows0 + p * P
                    eng.dma_start(out=y[r:r + P, col0:col0 + NCH],
                                  in_=y_sb[:])

                for i, name in enumerate(PROD):
                    pss = []
                    for p in range(2):
                        if i == NP_ - 1 and p == 1:
                            break  # handled below as two column halves
                        ps2 = ps_pool.tile([P, NCH], F32, tag="ps",
                                           name=f"ps_{name}_{p}")
                        biased = name in ("M1", "M5")
                        for kk in range(KS):
                            nc.tensor.matmul(
                                ps2[:],
                                a_op(name, kk, p),
                                w2_tiles[i * KS + kk][:],
                                start=(kk == 0),
                                stop=(kk == KS - 1) and not biased,
                            )
                        if biased:
                            off = 0 if name == "M1" else NCH
                            nc.tensor.matmul(
                                ps2[:], ones_sb[:], b2v_sb[:, off:off + NCH],
                                start=False, stop=True,
                            )
                        pss.append(ps2)

                    if name == "M1":
                        for p in range(2):
                            nc.scalar.activation(m1_sb[:, p, :], pss[p][:],
                                                 COPY)
                    elif name == "M5":
                        for p in range(2):
                            nc.scalar.activation(m5_sb[:, p, :], pss[p][:],
                                                 COPY)
                        for p in range(2):
                            nc.vector.tensor_add(u4_sb[:, p, :],
                                                 u2_sb[:, p, :],
                                                 m5_sb[:, p, :])
                    elif name == "M6":
                        for p in range(2):
                            nc.vector.tensor_add(u2_sb[:, p, :],
                                                 m1_sb[:, p, :], pss[p][:])
                    elif name == "M7":
                        for p in range(2):
                            nc.vector.tensor_add(u3_sb[:, p, :],
                                                 u2_sb[:, p, :], pss[p][:])
                        # C22 = U3 + M5 -> y[256:512, 512:1024]
                        for p in range(2):
                            emit_y(u3_sb[:, p, :], m5_sb[:, p, :], False,
                                   HB, NCH, p)
                    elif name == "M2":
                        # C11 = M1 + M2 -> y[0:256, 0:512]
                        for p in range(2):
                            emit_y(m1_sb[:, p, :], pss[p][:], False,
                                   0, 0, p)
                    elif name == "M3":
                        # C12 = U4 + M3 -> y[0:256, 512:1024]
                        for p in range(2):
                            emit_y(u4_sb[:, p, :], pss[p][:], False,
                                   0, NCH, p)
                    elif name == "M4":
                        # C21 = U3 - M4 -> y[256:512, 0:512].  ptile 0 is
                        # full-width; ptile 1 runs as two column halves in
                        # separate PSUM banks so the first half's evict+DMA
                        # hides under the second half's matmuls.
                        emit_y(u3_sb[:, 0, :], pss[0][:], True, HB, 0, 0,
                               via_sp=True)
                        for c0, hc in ((0, 3 * NCH // 4), (3 * NCH // 4,
                                                           NCH // 4)):
                            psh = ps_pool.tile([P, NCH], F32, tag="ps",
                                               name=f"ps_M4h{c0}")
                            for kk in range(KS):
                                nc.tensor.matmul(
                                    psh[:, 0:hc],
                                    a_op(name, kk, 1),
                                    w2_tiles[i * KS + kk][:, c0:c0 + hc],
                                    start=(kk == 0),
                                    stop=(kk == KS - 1),
                                )
                            y_sb = y_pool.tile([P, hc], BF16, tag="yh",
                                               name="y_sbh")
                            nc.vector.tensor_sub(
                                y_sb[:], u3_sb[:, 1, c0:c0 + hc],
                                psh[:, 0:hc],
                            )
                            nc.sync.dma_start(
                                out=y[HB + P:BSH, c0:c0 + hc], in_=y_sb[:],
                            )
                    mprev[name] = pss
    nc.compile()
    return nc


def _prep_shared(W1, b1, W2, b2):
    W1 = np.ascontiguousarray(W1, dtype=np.float32)
    # w1r[m, p, k*128+c] = W1[k*128+p, m*128+c]
    w1r = np.ascontiguousarray(
        W1.reshape(KT1, P, MT1, P).transpose(2, 1, 0, 3)
    ).reshape(MT1, P, N_IN).astype(BF)
    b1t = np.ascontiguousarray(
        np.asarray(b1, dtype=np.float32).reshape(MT1, P).T
    )
    W2 = np.ascontiguousarray(W2, dtype=np.float32)
    B11, B12 = W2[:2048, :NCH], W2[:2048, NCH:]
    B21, B22 = W2[2048:, :NCH], W2[2048:, NCH:]
    T1 = B12 - B11
    T2 = B22 - T1
    T3 = B22 - B12
    T4 = T2 - B21
    bmats = {"M1": B11, "M6": T2, "M5": T1, "M7": T3, "M2": B21,
             "M3": B22, "M4": T4}
    w2s = np.stack([bmats[nm].reshape(KS, P, NCH) for nm in PROD]
                   ).astype(BF)
    b2 = np.asarray(b2, dtype=np.float32)
    b2v = np.concatenate([b2[:NCH], b2[NCH:] - b2[:NCH]])[None, :].astype(BF)
    return w1r, b1t, w2s, b2v


def kernel(x, W1, b1, W2, b2):
    x = np.ascontiguousarray(x, dtype=np.float32)
    w1r, b1t, w2s, b2v = _prep_shared(W1, b1, W2, b2)

    in_maps = []
    for i in range(N_CORES):
        xs = x[i * BSH:(i + 1) * BSH, :].T.astype(BF)          # [1024, 512]
        xtr_i = np.ascontiguousarray(
            xs.reshape(KT1, P, BSH).transpose(1, 0, 2)
        ).reshape(P, KT1 * BSH)
        in_maps.append(
            {"xtr": xtr_i, "w1r": w1r, "w2s": w2s, "b1t": b1t, "b2v": b2v}
        )

    nc = build_nc()
    res = run_bass_kernel_spmd(nc, in_maps, list(range(N_CORES)))
    y = np.concatenate(
        [np.asarray(res.results[i]["y"]) for i in range(N_CORES)], axis=0
    )
    return y.astype(np.float32)


if __name__ == "__main__":
    rng = np.random.default_rng(0)
    x = rng.standard_normal((B, N_IN), dtype=np.float32)
    W1 = rng.standard_normal((N_IN, N_HID), dtype=np.float32) / 32
    b1 = rng.standard_normal((N_HID,), dtype=np.float32) / 32
    W2 = rng.standard_normal((N_HID, N_OUT), dtype=np.float32) / 64
    b2 = rng.standard_normal((N_OUT,), dtype=np.float32) / 64
    y = kernel(x, W1, b1, W2, b2)
    h = np.maximum(x @ W1 + b1, 0)
    y_ref = h @ W2 + b2
    err = np.linalg.norm(y - y_ref) / np.linalg.norm(y_ref)
    print("rel_l2:", err)


# revision 23
# speedup vs baseline: 1.0055x; 1.0022x over previous
"""Trainium2 Bass kernel for nn_LiveNet (2-layer MLP: relu(x@W1+b1)@W2+b2).

Sharding: pure data-parallel over batch across 8 NeuronCores (no
collectives).  Each core computes y_i = relu(x_i @ W1 + b1) @ W2 + b2 for
its 512-row batch shard.

All matmul operands are bf16 (host-cast): same 1 cycle/row PE rate as
float32r but half the HBM traffic, so DMA stays well below the PE
roofline.  Accumulation is fp32 in PSUM.

GEMM1 (hT = relu(W1.T @ xT + b1)) is classical: per 128-row hid m-tile,
8 k-matmuls accumulate in PSUM and ScalarE applies bias+ReLU on evict to
a resident bf16 hT.  Warmup matmuls on a zeroed tile ramp the PE p-state
(full 2.4 GHz needs 3us of continuous execution) while startup DMAs land.

GEMM2 (y = h @ W2 + b2) uses one level of Strassen-Winograd: 7 products
of half-size blocks instead of 8, cutting PE time by 1/8.  The W2-side
block combinations (T1..T4) are precomputed on host and streamed; the
h-side combinations (S1..S4) run on the otherwise-idle VectorE as hT
tiles appear; the C-side recombination runs on VectorE against PSUM
directly (only M1/M5 are evicted, being multiply-used), with y writeback
staggered product-by-product.  b2 is injected via rank-1 PE updates into
M1 and M5 (every C block inherits exactly M1 + [C12/C22: M5's delta]).
y is written bf16 and upcast on host.
"""

import os
import sys

import numpy as np

for _p in ("/opt/trn_rl_repo", "/root/.axon_site/_ro/trn_rl_repo"):
    if os.path.isdir(_p) and _p not in sys.path:
        sys.path.append(_p)

import ml_dtypes

import concourse.bacc as bacc
import concourse.bass as bass
import concourse.tile as tile
from concourse import mybir
from concourse.bass_utils import run_bass_kernel_spmd

N_CORES = 8
B, N_IN, N_HID, N_OUT = 4096, 1024, 4096, 1024
BSH = B // N_CORES          # 512 batch rows per core
P = 128                     # SBUF partitions
KT1 = N_IN // P             # 8  k-tiles in GEMM1
MT1 = N_HID // P            # 32 m-tiles (hid) in GEMM1
KS = MT1 // 2               # 16 k-tiles per Strassen half (hid 2048)
NCH = 512                   # moving free dim per matmul / N_OUT half
HB = 256                    # batch half per Strassen block

F32 = mybir.dt.float32
BF16 = mybir.dt.bfloat16
RELU = mybir.ActivationFunctionType.Relu
COPY = mybir.ActivationFunctionType.Copy
BF = ml_dtypes.bfloat16

WARMUP = int(os.environ.get("K_WARMUP", "30"))   # 128-row p-state ramp matmuls

# Strassen product order: M1, M6, M5, M7, M2, M3, M4 (multiply-used products
# first so later products can be consumed straight out of PSUM).
PROD = ["M1", "M6", "M5", "M7", "M2", "M3", "M4"]
NP_ = len(PROD)


def build_nc(reps=1):
    nc = bacc.Bacc("TRN2", target_bir_lowering=False, debug=False,
                   num_devices=N_CORES)

    # xtr[p, k*512+j] = x[batch j, k*128+p]: per-partition contiguous.
    xtr = nc.declare_dram_parameter("xtr", [P, KT1 * BSH], BF16, isOutput=False)
    w1r = nc.declare_dram_parameter("w1r", [MT1, P, N_IN], BF16, isOutput=False)
    # W2-side Strassen operands, one [2048, 512] matrix per product.
    w2s = nc.declare_dram_parameter("w2s", [NP_, KS, P, NCH], BF16,
                                    isOutput=False)
    b1t = nc.declare_dram_parameter("b1t", [P, MT1], F32, isOutput=False)
    # b2v = [b2[:512], b2[512:]-b2[:512]] for the M1/M5 rank-1 bias updates.
    b2v = nc.declare_dram_parameter("b2v", [1, N_OUT], BF16, isOutput=False)
    y = nc.declare_dram_parameter("y", [BSH, N_OUT], BF16, isOutput=True)

    with tile.TileContext(nc) as tc:
        with (
            tc.tile_pool(name="const", bufs=1) as const,
            tc.tile_pool(name="xt", bufs=1) as xt_pool,
            tc.tile_pool(name="ht", bufs=1) as ht_pool,
            tc.tile_pool(name="w1", bufs=6) as w1_pool,
            tc.tile_pool(name="w2", bufs=40) as w2_pool,
            tc.tile_pool(name="scmb", bufs=1) as s_pool,
            tc.tile_pool(name="uacc", bufs=1) as u_pool,
            tc.tile_pool(name="yout", bufs=4) as y_pool,
            tc.tile_pool(name="ps", bufs=4, space=bass.MemorySpace.PSUM) as ps_pool,
            tc.tile_pool(name="psb", bufs=4, space=bass.MemorySpace.PSUM) as psb_pool,
        ):
            # --- PE warmup: ramp the p-state while startup DMAs land ---
            wz = const.tile([P, P], BF16)
            nc.vector.memset(wz[:], 0.0)
            wps = ps_pool.tile([P, NCH], F32, tag="ps", name="wps")
            for i in range(WARMUP):
                nc.tensor.matmul(wps[:, 0:P], wz[:], wz[:],
                                 start=(i == 0), stop=(i == WARMUP - 1))

            # --- startup DMAs, critical-path first.  xt k-tiles stream on
            # SP while the first W1 m-tiles issue in parallel from ACT (its
            # first evict comes much later). ---
            xt_sb = xt_pool.tile([P, KT1, BSH], BF16, tag="xt", name="xt_sb")
            w1_head = []
            for m in range(4):
                w1_sb = w1_pool.tile([P, N_IN], BF16, tag="w1", name="w1_sb")
                if m == 0:
                    # halves: the first matmul only waits on k-tiles 0..3
                    nc.scalar.dma_start(out=w1_sb[:, 0:N_IN // 2],
                                        in_=w1r[m, :, 0:N_IN // 2])
                    nc.scalar.dma_start(out=w1_sb[:, N_IN // 2:N_IN],
                                        in_=w1r[m, :, N_IN // 2:N_IN])
                else:
                    nc.scalar.dma_start(out=w1_sb[:], in_=w1r[m])
                w1_head.append(w1_sb)
            for q in range(4):
                nc.sync.dma_start(
                    out=xt_sb[:, 2 * q:2 * q + 2, :],
                    in_=xtr[:, 2 * q * BSH:(2 * q + 2) * BSH])

            b1_sb = const.tile([P, MT1], F32)
            nc.gpsimd.dma_start(out=b1_sb[:], in_=b1t[:])
            b2v_sb = const.tile([1, N_OUT], BF16)
            nc.gpsimd.dma_start(out=b2v_sb[:], in_=b2v[:])
            ones_sb = const.tile([1, P], BF16)
            nc.vector.memset(ones_sb[:], 1.0)
            # GEMM2's rank-1 bias updates run now, into dedicated PSUM
            # banks that stay open (start=True, no stop) until their
            # products accumulate on top during GEMM2 — this moves their
            # PE cost into the warmup window.
            ps_bias = {}
            for nm, off in (("M1", 0), ("M5", NCH)):
                ps_bias[nm] = []
                for p in range(2):
                    psb = psb_pool.tile([P, NCH], F32, tag="psb",
                                        name=f"psb_{nm}_{p}")
                    nc.tensor.matmul(psb[:], ones_sb[:],
                                     b2v_sb[:, off:off + NCH],
                                     start=True, stop=False)
                    ps_bias[nm].append(psb)
            # Prime ACT with the bias-load DMA wait so later evict
            # instructions (which already wait on the PE sem) don't exceed
            # the per-instruction sync-wait budget in walrus codegen.
            prime1 = const.tile([P, 1], F32)
            nc.scalar.activation(prime1[:], b1_sb[:, 0:1], COPY)
            prime2 = const.tile([1, 1], BF16)
            nc.vector.tensor_copy(prime2[:], b2v_sb[:, 0:1])

            for rep in range(reps):
                # hT resident: [128, 32, 512] bf16; hT[p, m, j] =
                # h[batch j, hid m*128+p].
                ht_sb = ht_pool.tile([P, MT1, BSH], BF16, tag="ht",
                                     name="ht_sb")
                # h-side Strassen combos: S1=A21+A22, S2=S1-A11,
                # S3=A11-A21, S4=A12-S2; each [128, 16, 256] bf16.
                s1 = s_pool.tile([P, KS, HB], BF16, tag="s1", name="s1")
                s2 = s_pool.tile([P, KS, HB], BF16, tag="s2", name="s2")
                s3 = s_pool.tile([P, KS, HB], BF16, tag="s3", name="s3")
                s4 = s_pool.tile([P, KS, HB], BF16, tag="s4", name="s4")

                # W2-side product operand stream (pool-throttled JIT).
                w2_tiles = []
                w2_iss = 0

                def issue_w2(cnt):
                    nonlocal w2_iss
                    for _ in range(cnt):
                        if w2_iss >= NP_ * KS:
                            return
                        i, kk = w2_iss // KS, w2_iss % KS
                        t = w2_pool.tile([P, NCH], BF16, tag="w2",
                                         name="w2t")
                        nc.sync.dma_start(out=t[:], in_=w2s[i, kk])
                        w2_tiles.append(t)
                        w2_iss += 1

                # ---- GEMM1: hT = relu(W1.T-tiled @ xT + b1) ----
                for m in range(MT1):
                    if m < 4:
                        w1_sb = w1_head[m]
                    else:
                        w1_sb = w1_pool.tile([P, N_IN], BF16, tag="w1",
                                             name="w1_sb")
                        nc.sync.dma_start(out=w1_sb[:], in_=w1r[m])
                    if rep == 0 and m >= 1:
                        issue_w2(2)
                    ps = ps_pool.tile([P, BSH], F32, tag="ps", name="ps")
                    for k in range(KT1):
                        nc.tensor.matmul(
                            ps[:],
                            w1_sb[:, k * P:(k + 1) * P],
                            xt_sb[:, k, :],
                            start=(k == 0),
                            stop=(k == KT1 - 1),
                        )
                    nc.scalar.activation(
                        ht_sb[:, m, :], ps[:], RELU, bias=b1_sb[:, m:m + 1],
                    )
                    if m >= KS:
                        kk = m - KS
                        a11 = ht_sb[:, kk, 0:HB]
                        a21 = ht_sb[:, kk, HB:BSH]
                        a12 = ht_sb[:, m, 0:HB]
                        a22 = ht_sb[:, m, HB:BSH]
                        nc.vector.tensor_add(s1[:, kk, :], a21, a22)
                        nc.vector.tensor_sub(s2[:, kk, :], s1[:, kk, :], a11)
                        nc.vector.tensor_sub(s4[:, kk, :], a12, s2[:, kk, :])
                        nc.vector.tensor_sub(s3[:, kk, :], a11, a21)
                issue_w2(NP_ * KS - w2_iss)

                # ---- GEMM2: one level of Strassen-Winograd ----
                # A-side operand for product name, k-tile kk, batch ptile p.
                def a_op(name, kk, p):
                    j0 = p * P
                    if name == "M1":          # A11
                        return ht_sb[:, kk, j0:j0 + P]
                    if name == "M2":          # A12
                        return ht_sb[:, KS + kk, j0:j0 + P]
                    if name == "M4":          # A22
                        return ht_sb[:, KS + kk, HB + j0:HB + j0 + P]
                    s = {"M6": s2, "M5": s1, "M7": s3, "M3": s4}[name]
                    return s[:, kk, j0:j0 + P]

                m1_sb = u_pool.tile([P, 2, NCH], F32, tag="m1", name="m1_sb")
                m5_sb = u_pool.tile([P, 2, NCH], F32, tag="m5", name="m5_sb")
                u2_sb = u_pool.tile([P, 2, NCH], F32, tag="u2", name="u2_sb")
                u3_sb = u_pool.tile([P, 2, NCH], F32, tag="u3", name="u3_sb")
                u4_sb = u_pool.tile([P, 2, NCH], F32, tag="u4", name="u4_sb")

                mprev = {}

                def emit_y(tile_src0, tile_src1, sub, rows0, col0, p,
                           via_sp=False):
                    """y[rows0+p*128 .., col0:col0+512] = src0 +/- src1."""
                    y_sb = y_pool.tile([P, NCH], BF16, tag="y", name="y_sb")
                    if sub:
                        nc.vector.tensor_sub(y_sb[:], tile_src0, tile_src1)
                    else:
                        nc.vector.tensor_add(y_sb[:], tile_src0, tile_src1)
                    eng = nc.sync if via_sp else nc.scalar
                    r = rows0 + p * P
                    eng.dma_start(out=y[r:r + P, col0:col0 + NCH],
                                  in_=y_sb[:])

                for i, name in enumerate(PROD):
                    pss = []
                    for p in range(2):
                        if i == NP_ - 1 and p == 1:
                            break  # handled below as two column halves
                        biased = name in ("M1", "M5")
                        if biased:
                            ps2 = ps_bias[name][p]
                        else:
                            ps2 = ps_pool.tile([P, NCH], F32, tag="ps",
                                               name=f"ps_{name}_{p}")
                        for kk in range(KS):
                            nc.tensor.matmul(
                                ps2[:],
                                a_op(name, kk, p),
                                w2_tiles[i * KS + kk][:],
                                start=(kk == 0) and not biased,
                                stop=(kk == KS - 1),
                            )
                        pss.append(ps2)

                    if name == "M1":
                        for p in range(2):
                            nc.scalar.activation(m1_sb[:, p, :], pss[p][:],
                                                 COPY)
                    elif name == "M5":
                        for p in range(2):
                            nc.scalar.activation(m5_sb[:, p, :], pss[p][:],
                                                 COPY)
                        for p in range(2):
                            nc.vector.tensor_add(u4_sb[:, p, :],
                                                 u2_sb[:, p, :],
                                                 m5_sb[:, p, :])
                    elif name == "M6":
                        for p in range(2):
                            nc.vector.tensor_add(u2_sb[:, p, :],
                                                 m1_sb[:, p, :], pss[p][:])
                    elif name == "M7":
                        for p in range(2):
                            nc.vector.tensor_add(u3_sb[:, p, :],
                                                 u2_sb[:, p, :], pss[p][:])
                        # C22 = U3 + M5 -> y[256:512, 512:1024]
                        for p in range(2):
                            emit_y(u3_sb[:, p, :], m5_sb[:, p, :], False,
                                   HB, NCH, p)
                    elif name == "M2":
                        # C11 = M1 + M2 -> y[0:256, 0:512]
                        for p in range(2):
                            emit_y(m1_sb[:, p, :], pss[p][:], False,
                                   0, 0, p)
                    elif name == "M3":
                        # C12 = U4 + M3 -> y[0:256, 512:1024]
                        for p in range(2):
                            emit_y(u4_sb[:, p, :], pss[p][:], False,
                                   0, NCH, p)
                    elif name == "M4":
                        # C21 = U3 - M4 -> y[256:512, 0:512].  ptile 0 is
                        # full-width; ptile 1 runs as two column halves in
                        # separate PSUM banks so the first half's evict+DMA
                        # hides under the second half's matmuls.
                        emit_y(u3_sb[:, 0, :], pss[0][:], True, HB, 0, 0,
                               via_sp=True)
                        for c0, hc in ((0, 3 * NCH // 4), (3 * NCH // 4,
                                                           NCH // 4)):
                            psh = ps_pool.tile([P, NCH], F32, tag="ps",
                                               name=f"ps_M4h{c0}")
                            for kk in range(KS):
                                nc.tensor.matmul(
                                    psh[:, 0:hc],
                                    a_op(name, kk, 1),
                                    w2_tiles[i * KS + kk][:, c0:c0 + hc],
                                    start=(kk == 0),
                                    stop=(kk == KS - 1),
                                )
                            y_sb = y_pool.tile([P, hc], BF16, tag="yh",
                                               name="y_sbh")
                            nc.vector.tensor_sub(
                                y_sb[:], u3_sb[:, 1, c0:c0 + hc],
                                psh[:, 0:hc],
                            )
                            nc.sync.dma_start(
                                out=y[HB + P:BSH, c0:c0 + hc], in_=y_sb[:],
                            )
                    mprev[name] = pss
    nc.compile()
    return nc


def _prep_shared(W1, b1, W2, b2):
    W1 = np.ascontiguousarray(W1, dtype=np.float32)
    # w1r[m, p, k*128+c] = W1[k*128+p, m*128+c]
    w1r = np.ascontiguousarray(
        W1.reshape(KT1, P, MT1, P).transpose(2, 1, 0, 3)
    ).reshape(MT1, P, N_IN).astype(BF)
    b1t = np.ascontiguousarray(
        np.asarray(b1, dtype=np.float32).reshape(MT1, P).T
    )
    W2 = np.ascontiguousarray(W2, dtype=np.float32)
    B11, B12 = W2[:2048, :NCH], W2[:2048, NCH:]
    B21, B22 = W2[2048:, :NCH], W2[2048:, NCH:]
    T1 = B12 - B11
    T2 = B22 - T1
    T3 = B22 - B12
    T4 = T2 - B21
    bmats = {"M1": B11, "M6": T2, "M5": T1, "M7": T3, "M2": B21,
             "M3": B22, "M4": T4}
    w2s = np.stack([bmats[nm].reshape(KS, P, NCH) for nm in PROD]
                   ).astype(BF)
    b2 = np.asarray(b2, dtype=np.float32)
    b2v = np.concatenate([b2[:NCH], b2[NCH:] - b2[:NCH]])[None, :].astype(BF)
    return w1r, b1t, w2s, b2v


def kernel(x, W1, b1, W2, b2):
    x = np.ascontiguousarray(x, dtype=np.float32)
    w1r, b1t, w2s, b2v = _prep_shared(W1, b1, W2, b2)

    in_maps = []
    for i in range(N_CORES):
        xs = x[i * BSH:(i + 1) * BSH, :].T.astype(BF)          # [1024, 512]
        xtr_i = np.ascontiguousarray(
            xs.reshape(KT1, P, BSH).transpose(1, 0, 2)
        ).reshape(P, KT1 * BSH)
        in_maps.append(
            {"xtr": xtr_i, "w1r": w1r, "w2s": w2s, "b1t": b1t, "b2v": b2v}
        )

    nc = build_nc()
    res = run_bass_kernel_spmd(nc, in_maps, list(range(N_CORES)))
    y = np.concatenate(
        [np.asarray(res.results[i]["y"]) for i in range(N_CORES)], axis=0
    )
    return y.astype(np.float32)


if __name__ == "__main__":
    rng = np.random.default_rng(0)
    x = rng.standard_normal((B, N_IN), dtype=np.float32)
    W1 = rng.standard_normal((N_IN, N_HID), dtype=np.float32) / 32
    b1 = rng.standard_normal((N_HID,), dtype=np.float32) / 32
    W2 = rng.standard_normal((N_HID, N_OUT), dtype=np.float32) / 64
    b2 = rng.standard_normal((N_OUT,), dtype=np.float32) / 64
    y = kernel(x, W1, b1, W2, b2)
    h = np.maximum(x @ W1 + b1, 0)
    y_ref = h @ W2 + b2
    err = np.linalg.norm(y - y_ref) / np.linalg.norm(y_ref)
    print("rel_l2:", err)


# revision 24
# speedup vs baseline: 1.0075x; 1.0020x over previous
"""Trainium2 Bass kernel for nn_LiveNet (2-layer MLP: relu(x@W1+b1)@W2+b2).

Sharding: pure data-parallel over batch across 8 NeuronCores (no
collectives).  Each core computes y_i = relu(x_i @ W1 + b1) @ W2 + b2 for
its 512-row batch shard.

All matmul operands are bf16 (host-cast): same 1 cycle/row PE rate as
float32r but half the HBM traffic, so DMA stays well below the PE
roofline.  Accumulation is fp32 in PSUM.

GEMM1 (hT = relu(W1.T @ xT + b1)) is classical: per 128-row hid m-tile,
8 k-matmuls accumulate in PSUM and ScalarE applies bias+ReLU on evict to
a resident bf16 hT.  Warmup matmuls on a zeroed tile ramp the PE p-state
(full 2.4 GHz needs 3us of continuous execution) while startup DMAs land.

GEMM2 (y = h @ W2 + b2) uses one level of Strassen-Winograd: 7 products
of half-size blocks instead of 8, cutting PE time by 1/8.  The W2-side
block combinations (T1..T4) are precomputed on host and streamed; the
h-side combinations (S1..S4) run on the otherwise-idle VectorE as hT
tiles appear; the C-side recombination runs on VectorE against PSUM
directly (only M1/M5 are evicted, being multiply-used), with y writeback
staggered product-by-product.  b2 is injected via rank-1 PE updates into
M1 and M5 (every C block inherits exactly M1 + [C12/C22: M5's delta]).
y is written bf16 and upcast on host.
"""

import os
import sys

import numpy as np

for _p in ("/opt/trn_rl_repo", "/root/.axon_site/_ro/trn_rl_repo"):
    if os.path.isdir(_p) and _p not in sys.path:
        sys.path.append(_p)

import ml_dtypes

import concourse.bacc as bacc
import concourse.bass as bass
import concourse.tile as tile
from concourse import mybir
from concourse.bass_utils import run_bass_kernel_spmd

N_CORES = 8
B, N_IN, N_HID, N_OUT = 4096, 1024, 4096, 1024
BSH = B // N_CORES          # 512 batch rows per core
P = 128                     # SBUF partitions
KT1 = N_IN // P             # 8  k-tiles in GEMM1
MT1 = N_HID // P            # 32 m-tiles (hid) in GEMM1
KS = MT1 // 2               # 16 k-tiles per Strassen half (hid 2048)
NCH = 512                   # moving free dim per matmul / N_OUT half
HB = 256                    # batch half per Strassen block

F32 = mybir.dt.float32
BF16 = mybir.dt.bfloat16
RELU = mybir.ActivationFunctionType.Relu
COPY = mybir.ActivationFunctionType.Copy
BF = ml_dtypes.bfloat16

WARMUP = int(os.environ.get("K_WARMUP", "22"))   # 128-row p-state ramp matmuls

# Strassen product order: M1, M6, M5, M7, M2, M3, M4 (multiply-used products
# first so later products can be consumed straight out of PSUM).
PROD = ["M1", "M6", "M5", "M7", "M2", "M3", "M4"]
NP_ = len(PROD)


def build_nc(reps=1):
    nc = bacc.Bacc("TRN2", target_bir_lowering=False, debug=False,
                   num_devices=N_CORES)

    # xtr[p, k*512+j] = x[batch j, k*128+p]: per-partition contiguous.
    xtr = nc.declare_dram_parameter("xtr", [P, KT1 * BSH], BF16, isOutput=False)
    w1r = nc.declare_dram_parameter("w1r", [MT1, P, N_IN], BF16, isOutput=False)
    # W2-side Strassen operands, one [2048, 512] matrix per product.
    w2s = nc.declare_dram_parameter("w2s", [NP_, KS, P, NCH], BF16,
                                    isOutput=False)
    b1t = nc.declare_dram_parameter("b1t", [P, MT1], F32, isOutput=False)
    # b2v = [b2[:512], b2[512:]-b2[:512]] for the M1/M5 rank-1 bias updates.
    b2v = nc.declare_dram_parameter("b2v", [1, N_OUT], BF16, isOutput=False)
    y = nc.declare_dram_parameter("y", [BSH, N_OUT], BF16, isOutput=True)

    with tile.TileContext(nc) as tc:
        with (
            tc.tile_pool(name="const", bufs=1) as const,
            tc.tile_pool(name="xt", bufs=1) as xt_pool,
            tc.tile_pool(name="ht", bufs=1) as ht_pool,
            tc.tile_pool(name="w1", bufs=6) as w1_pool,
            tc.tile_pool(name="w2", bufs=40) as w2_pool,
            tc.tile_pool(name="scmb", bufs=1) as s_pool,
            tc.tile_pool(name="uacc", bufs=1) as u_pool,
            tc.tile_pool(name="yout", bufs=4) as y_pool,
            tc.tile_pool(name="ps", bufs=4, space=bass.MemorySpace.PSUM) as ps_pool,
            tc.tile_pool(name="psb", bufs=4, space=bass.MemorySpace.PSUM) as psb_pool,
        ):
            # --- PE warmup: ramp the p-state while startup DMAs land ---
            wz = const.tile([P, P], BF16)
            nc.vector.memset(wz[:], 0.0)
            wps = ps_pool.tile([P, NCH], F32, tag="ps", name="wps")
            for i in range(WARMUP):
                nc.tensor.matmul(wps[:, 0:P], wz[:], wz[:],
                                 start=(i == 0), stop=(i == WARMUP - 1))

            # --- startup DMAs, critical-path first.  xt k-tiles stream on
            # SP while the first W1 m-tiles issue in parallel from ACT (its
            # first evict comes much later). ---
            xt_sb = xt_pool.tile([P, KT1, BSH], BF16, tag="xt", name="xt_sb")
            w1_head = []
            for m in range(4):
                w1_sb = w1_pool.tile([P, N_IN], BF16, tag="w1", name="w1_sb")
                if m == 0:
                    # halves: the first matmul only waits on k-tiles 0..3
                    nc.scalar.dma_start(out=w1_sb[:, 0:N_IN // 2],
                                        in_=w1r[m, :, 0:N_IN // 2])
                    nc.scalar.dma_start(out=w1_sb[:, N_IN // 2:N_IN],
                                        in_=w1r[m, :, N_IN // 2:N_IN])
                else:
                    nc.scalar.dma_start(out=w1_sb[:], in_=w1r[m])
                w1_head.append(w1_sb)
            for q in range(4):
                nc.sync.dma_start(
                    out=xt_sb[:, 2 * q:2 * q + 2, :],
                    in_=xtr[:, 2 * q * BSH:(2 * q + 2) * BSH])

            b1_sb = const.tile([P, MT1], F32)
            nc.gpsimd.dma_start(out=b1_sb[:], in_=b1t[:])
            b2v_sb = const.tile([1, N_OUT], BF16)
            nc.gpsimd.dma_start(out=b2v_sb[:], in_=b2v[:])
            ones_sb = const.tile([1, P], BF16)
            nc.vector.memset(ones_sb[:], 1.0)
            # GEMM2's rank-1 bias updates run now, into dedicated PSUM
            # banks that stay open (start=True, no stop) until their
            # products accumulate on top during GEMM2 — this moves their
            # PE cost into the warmup window.
            ps_bias = {}
            for nm, off in (("M1", 0), ("M5", NCH)):
                ps_bias[nm] = []
                for p in range(2):
                    psb = psb_pool.tile([P, NCH], F32, tag="psb",
                                        name=f"psb_{nm}_{p}")
                    nc.tensor.matmul(psb[:], ones_sb[:],
                                     b2v_sb[:, off:off + NCH],
                                     start=True, stop=False)
                    ps_bias[nm].append(psb)
            # Prime ACT with the bias-load DMA wait so later evict
            # instructions (which already wait on the PE sem) don't exceed
            # the per-instruction sync-wait budget in walrus codegen.
            prime1 = const.tile([P, 1], F32)
            nc.scalar.activation(prime1[:], b1_sb[:, 0:1], COPY)
            prime2 = const.tile([1, 1], BF16)
            nc.vector.tensor_copy(prime2[:], b2v_sb[:, 0:1])

            for rep in range(reps):
                # hT resident: [128, 32, 512] bf16; hT[p, m, j] =
                # h[batch j, hid m*128+p].
                ht_sb = ht_pool.tile([P, MT1, BSH], BF16, tag="ht",
                                     name="ht_sb")
                # h-side Strassen combos: S1=A21+A22, S2=S1-A11,
                # S3=A11-A21, S4=A12-S2; each [128, 16, 256] bf16.
                s1 = s_pool.tile([P, KS, HB], BF16, tag="s1", name="s1")
                s2 = s_pool.tile([P, KS, HB], BF16, tag="s2", name="s2")
                s3 = s_pool.tile([P, KS, HB], BF16, tag="s3", name="s3")
                s4 = s_pool.tile([P, KS, HB], BF16, tag="s4", name="s4")

                # W2-side product operand stream (pool-throttled JIT).
                w2_tiles = []
                w2_iss = 0

                def issue_w2(cnt):
                    nonlocal w2_iss
                    for _ in range(cnt):
                        if w2_iss >= NP_ * KS:
                            return
                        i, kk = w2_iss // KS, w2_iss % KS
                        t = w2_pool.tile([P, NCH], BF16, tag="w2",
                                         name="w2t")
                        nc.sync.dma_start(out=t[:], in_=w2s[i, kk])
                        w2_tiles.append(t)
                        w2_iss += 1

                # ---- GEMM1: hT = relu(W1.T-tiled @ xT + b1) ----
                for m in range(MT1):
                    if m < 4:
                        w1_sb = w1_head[m]
                    else:
                        w1_sb = w1_pool.tile([P, N_IN], BF16, tag="w1",
                                             name="w1_sb")
                        nc.sync.dma_start(out=w1_sb[:], in_=w1r[m])
                    if rep == 0 and m >= 1:
                        issue_w2(2)
                    ps = ps_pool.tile([P, BSH], F32, tag="ps", name="ps")
                    for k in range(KT1):
                        nc.tensor.matmul(
                            ps[:],
                            w1_sb[:, k * P:(k + 1) * P],
                            xt_sb[:, k, :],
                            start=(k == 0),
                            stop=(k == KT1 - 1),
                        )
                    nc.scalar.activation(
                        ht_sb[:, m, :], ps[:], RELU, bias=b1_sb[:, m:m + 1],
                    )
                    if m >= KS:
                        kk = m - KS
                        a11 = ht_sb[:, kk, 0:HB]
                        a21 = ht_sb[:, kk, HB:BSH]
                        a12 = ht_sb[:, m, 0:HB]
                        a22 = ht_sb[:, m, HB:BSH]
                        nc.vector.tensor_add(s1[:, kk, :], a21, a22)
                        nc.vector.tensor_sub(s2[:, kk, :], s1[:, kk, :], a11)
                        nc.vector.tensor_sub(s4[:, kk, :], a12, s2[:, kk, :])
                        nc.vector.tensor_sub(s3[:, kk, :], a11, a21)
                issue_w2(NP_ * KS - w2_iss)

                # ---- GEMM2: one level of Strassen-Winograd ----
                # A-side operand for product name, k-tile kk, batch ptile p.
                def a_op(name, kk, p):
                    j0 = p * P
                    if name == "M1":          # A11
                        return ht_sb[:, kk, j0:j0 + P]
                    if name == "M2":          # A12
                        return ht_sb[:, KS + kk, j0:j0 + P]
                    if name == "M4":          # A22
                        return ht_sb[:, KS + kk, HB + j0:HB + j0 + P]
                    s = {"M6": s2, "M5": s1, "M7": s3, "M3": s4}[name]
                    return s[:, kk, j0:j0 + P]

                m1_sb = u_pool.tile([P, 2, NCH], F32, tag="m1", name="m1_sb")
                m5_sb = u_pool.tile([P, 2, NCH], F32, tag="m5", name="m5_sb")
                u2_sb = u_pool.tile([P, 2, NCH], F32, tag="u2", name="u2_sb")
                u3_sb = u_pool.tile([P, 2, NCH], F32, tag="u3", name="u3_sb")
                u4_sb = u_pool.tile([P, 2, NCH], F32, tag="u4", name="u4_sb")

                mprev = {}

                def emit_y(tile_src0, tile_src1, sub, rows0, col0, p,
                           via_sp=False):
                    """y[rows0+p*128 .., col0:col0+512] = src0 +/- src1."""
                    y_sb = y_pool.tile([P, NCH], BF16, tag="y", name="y_sb")
                    if sub:
                        nc.vector.tensor_sub(y_sb[:], tile_src0, tile_src1)
                    else:
                        nc.vector.tensor_add(y_sb[:], tile_src0, tile_src1)
                    eng = nc.sync if via_sp else nc.scalar
                    r = rows0 + p * P
                    eng.dma_start(out=y[r:r + P, col0:col0 + NCH],
                                  in_=y_sb[:])

                for i, name in enumerate(PROD):
                    pss = []
                    for p in range(2):
                        if i == NP_ - 1 and p == 1:
                            break  # handled below as two column halves
                        biased = name in ("M1", "M5")
                        if biased:
                            ps2 = ps_bias[name][p]
                        else:
                            ps2 = ps_pool.tile([P, NCH], F32, tag="ps",
                                               name=f"ps_{name}_{p}")
                        for kk in range(KS):
                            nc.tensor.matmul(
                                ps2[:],
                                a_op(name, kk, p),
                                w2_tiles[i * KS + kk][:],
                                start=(kk == 0) and not biased,
                                stop=(kk == KS - 1),
                            )
                        pss.append(ps2)

                    if name == "M1":
                        for p in range(2):
                            nc.scalar.activation(m1_sb[:, p, :], pss[p][:],
                                                 COPY)
                    elif name == "M5":
                        for p in range(2):
                            nc.scalar.activation(m5_sb[:, p, :], pss[p][:],
                                                 COPY)
                        for p in range(2):
                            nc.vector.tensor_add(u4_sb[:, p, :],
                                                 u2_sb[:, p, :],
                                                 m5_sb[:, p, :])
                    elif name == "M6":
                        for p in range(2):
                            nc.vector.tensor_add(u2_sb[:, p, :],
                                                 m1_sb[:, p, :], pss[p][:])
                    elif name == "M7":
                        for p in range(2):
                            nc.vector.tensor_add(u3_sb[:, p, :],
                                                 u2_sb[:, p, :], pss[p][:])
                        # C22 = U3 + M5 -> y[256:512, 512:1024]
                        for p in range(2):
                            emit_y(u3_sb[:, p, :], m5_sb[:, p, :], False,
                                   HB, NCH, p)
                    elif name == "M2":
                        # C11 = M1 + M2 -> y[0:256, 0:512]
                        for p in range(2):
                            emit_y(m1_sb[:, p, :], pss[p][:], False,
                                   0, 0, p)
                    elif name == "M3":
                        # C12 = U4 + M3 -> y[0:256, 512:1024]
                        for p in range(2):
                            emit_y(u4_sb[:, p, :], pss[p][:], False,
                                   0, NCH, p)
                    elif name == "M4":
                        # C21 = U3 - M4 -> y[256:512, 0:512].  ptile 0 is
                        # full-width; ptile 1 runs as two column halves in
                        # separate PSUM banks so the first half's evict+DMA
                        # hides under the second half's matmuls.
                        emit_y(u3_sb[:, 0, :], pss[0][:], True, HB, 0, 0,
                               via_sp=True)
                        for c0, hc in ((0, 3 * NCH // 4), (3 * NCH // 4,
                                                           NCH // 4)):
                            psh = ps_pool.tile([P, NCH], F32, tag="ps",
                                               name=f"ps_M4h{c0}")
                            for kk in range(KS):
                                nc.tensor.matmul(
                                    psh[:, 0:hc],
                                    a_op(name, kk, 1),
                                    w2_tiles[i * KS + kk][:, c0:c0 + hc],
                                    start=(kk == 0),
                                    stop=(kk == KS - 1),
                                )
                            y_sb = y_pool.tile([P, hc], BF16, tag="yh",
                                               name="y_sbh")
                            nc.vector.tensor_sub(
                                y_sb[:], u3_sb[:, 1, c0:c0 + hc],
                                psh[:, 0:hc],
                            )
                            nc.sync.dma_start(
                                out=y[HB + P:BSH, c0:c0 + hc], in_=y_sb[:],
                            )
                    mprev[name] = pss
    nc.compile()
    return nc


def _prep_shared(W1, b1, W2, b2):
    W1 = np.ascontiguousarray(W1, dtype=np.float32)
    # w1r[m, p, k*128+c] = W1[k*128+p, m*128+c]
    w1r = np.ascontiguousarray(
        W1.reshape(KT1, P, MT1, P).transpose(2, 1, 0, 3)
    ).reshape(MT1, P, N_IN).astype(BF)
    b1t = np.ascontiguousarray(
        np.asarray(b1, dtype=np.float32).reshape(MT1, P).T
    )
    W2 = np.ascontiguousarray(W2, dtype=np.float32)
    B11, B12 = W2[:2048, :NCH], W2[:2048, NCH:]
    B21, B22 = W2[2048:, :NCH], W2[2048:, NCH:]
    T1 = B12 - B11
    T2 = B22 - T1
    T3 = B22 - B12
    T4 = T2 - B21
    bmats = {"M1": B11, "M6": T2, "M5": T1, "M7": T3, "M2": B21,
             "M3": B22, "M4": T4}
    w2s = np.stack([bmats[nm].reshape(KS, P, NCH) for nm in PROD]
                   ).astype(BF)
    b2 = np.asarray(b2, dtype=np.float32)
    b2v = np.concatenate([b2[:NCH], b2[NCH:] - b2[:NCH]])[None, :].astype(BF)
    return w1r, b1t, w2s, b2v


def kernel(x, W1, b1, W2, b2):
    x = np.ascontiguousarray(x, dtype=np.float32)
    w1r, b1t, w2s, b2v = _prep_shared(W1, b1, W2, b2)

    in_maps = []
    for i in range(N_CORES):
        xs = x[i * BSH:(i + 1) * BSH, :].T.astype(BF)          # [1024, 512]
        xtr_i = np.ascontiguousarray(
            xs.reshape(KT1, P, BSH).transpose(1, 0, 2)
        ).reshape(P, KT1 * BSH)
        in_maps.append(
            {"xtr": xtr_i, "w1r": w1r, "w2s": w2s, "b1t": b1t, "b2v": b2v}
        )

    nc = build_nc()
    res = run_bass_kernel_spmd(nc, in_maps, list(range(N_CORES)))
    y = np.concatenate(
        [np.asarray(res.results[i]["y"]) for i in range(N_CORES)], axis=0
    )
    return y.astype(np.float32)


if __name__ == "__main__":
    rng = np.random.default_rng(0)
    x = rng.standard_normal((B, N_IN), dtype=np.float32)
    W1 = rng.standard_normal((N_IN, N_HID), dtype=np.float32) / 32
    b1 = rng.standard_normal((N_HID,), dtype=np.float32) / 32
    W2 = rng.standard_normal((N_HID, N_OUT), dtype=np.float32) / 64
    b2 = rng.standard_normal((N_OUT,), dtype=np.float32) / 64
    y = kernel(x, W1, b1, W2, b2)
    h = np.maximum(x @ W1 + b1, 0)
    y_ref = h @ W2 + b2
    err = np.linalg.norm(y - y_ref) / np.linalg.norm(y_ref)
    print("rel_l2:", err)
